# revision 21
# baseline (speedup 1.0000x reference)
"""Trainium2 Bass kernel for GQA MultiHeadAttention (nn_MultiHeadAttention_74028056314029).

Reference computation (fp32, single device):
    Q = x @ W_q.T; K = x @ W_k.T; V = x @ W_v.T   (H=32 query heads, KV=8, G=4)
    per query head: softmax(causal(Q_h K_h^T / sqrt(D))) @ V_h
    out = hidden @ W_o.T

Sharding (8 NeuronCores, tensor-parallel over heads):
    core c owns query heads [4c, 4c+4) == KV group c (1 KV head).
    Each core computes a full-shape partial of the output projection;
    the 8 partials are summed on the host - no on-device collective.

Device scheme (bf16 matmuls, fp32 PSUM):
    - host pre-transposes x / weight shards so every DMA is contiguous;
      weights go on the sync HWDGE ring, x chunks on the scalar ring so the
      first projection matmul starts ~4us in
    - Q.T/K.T live [head_dim, seq]; scores come out transposed [s, q]; the
      K.T lhsT is zero-padded to K=128 per head parity ([K.T;0] / [0;K.T]) so
      every score matmul streams the full PE array - half-array K=64 streams
      pin the HAM clock gate at 1.2 GHz
    - exp on ACT with the 1/sqrt(D) scale fused; causal diagonal masked by a
      bf16 upper-tri multiply; attn@V appends a ones column to V so the
      softmax denominator falls out of the same accumulation (row 64)
    - 1/den: the 1-lane den row bounces through DRAM into [128,16] so the DVE
      reciprocal is cheap, then a K=1 ones-matmul broadcasts it across
      partitions; the normalize multiply stays off the PE critical path
    - the Q m=1 projection strips are interleaved into attention pair 0 as
      dense accumulation bursts that keep the HAM clock gate warm
"""

import os
import numpy as np
import ml_dtypes

E, H, KVH, D = 2048, 32, 8, 64
B, C = 1, 2048
G = H // KVH              # 4 query heads per core
NCORES = 8
HD_C = G * D              # 256 query head dims per core
P = 128
NE = E // P               # 16 contraction chunks
NQ = C // P               # 16 sequence chunks
SW = 512                  # strip width (one PSUM bank of fp32)
NS = C // SW              # 4 strips

BF16 = ml_dtypes.bfloat16

_CACHE: dict = {}
LAST_RESULTS = None       # BassKernelResults of the most recent run (for profiling)
TRACE = bool(int(os.environ.get("KERNEL_TRACE", "0")))


def build_bass():
    import concourse.tile as tile
    import concourse.mybir as mybir
    from concourse import bacc
    from concourse.masks import make_identity

    bf16 = mybir.dt.bfloat16
    f32 = mybir.dt.float32
    AF = mybir.ActivationFunctionType

    nc = bacc.Bacc()
    xT = nc.declare_dram_parameter("xT", [E, C], bf16, isOutput=False)
    wqT = nc.declare_dram_parameter("wqT", [E, HD_C], bf16, isOutput=False)
    wkvT = nc.declare_dram_parameter("wkvT", [E, 2 * D], bf16, isOutput=False)
    woT = nc.declare_dram_parameter("woT", [HD_C, E], bf16, isOutput=False)
    tri = nc.declare_dram_parameter("tri", [P, P], bf16, isOutput=False)
    outp = nc.declare_dram_parameter("out_part", [C, E], f32, isOutput=True)
    scr_den = nc.dram_tensor("scr_den", [G, 1, C], f32)
    scr_rec = nc.dram_tensor("scr_rec", [G, 1, C], f32)

    with tile.TileContext(nc) as tc:
        with (
            tc.tile_pool(name="big", bufs=1) as big,
            tc.tile_pool(name="expp", bufs=8) as expp,
            tc.tile_pool(name="denp", bufs=2) as denp,
            tc.tile_pool(name="d128", bufs=2) as d128p,
            tc.tile_pool(name="recp", bufs=3) as recp,
            tc.tile_pool(name="htp", bufs=2) as htp,
            tc.tile_pool(name="outs", bufs=2) as outs,
            tc.tile_pool(name="ps", bufs=4, space="PSUM") as ps,
            tc.tile_pool(name="psx", bufs=2, space="PSUM") as psx,
            tc.tile_pool(name="psh", bufs=2, space="PSUM") as psh,
        ):
            # ---- persistent SBUF tensors ----
            x_sb = big.tile([P, NE, C], bf16)        # x.T: E on partitions
            wq_sb = big.tile([P, NE, HD_C], bf16)
            wkv_sb = big.tile([P, NE, 2 * D], bf16)  # [W_k | W_v] shard, transposed
            wo_sb = big.tile([P, 2, E], bf16)        # W_o shard transposed: hd on partitions
            tri_sb = big.tile([P, P], bf16)          # upper-tri ones (q>=s valid)
            ident = big.tile([P, P], bf16)
            ones_sb = big.tile([P, D], bf16)         # ones row for the K=1 PE broadcast
            kt_e = big.tile([P, C], bf16)            # [K.T ; 0] for even heads
            kt_o = big.tile([P, C], bf16)            # [0 ; K.T] for odd heads
            vt_sb = big.tile([P, C], bf16)           # V.T staged at partitions 64:128
            v_sb = big.tile([P, NQ, D + 1], bf16)    # V natural + ones column
            qt_sb = big.tile([P, 2, C], bf16)        # Q.T: head-dim on partitions
            hid_sb = big.tile([P, 2, C], bf16)       # hidden.T (raw, then normalized)

            # weights first on the sync ring; x chunks stream on the ACT ring
            nc.sync.dma_start(out=wkv_sb, in_=wkvT[:].rearrange("(eo p) m -> p eo m", p=P))
            nc.sync.dma_start(out=wq_sb, in_=wqT[:].rearrange("(eo p) m -> p eo m", p=P))
            nc.sync.dma_start(out=tri_sb, in_=tri[:])
            nc.sync.dma_start(out=wo_sb, in_=woT[:].rearrange("(ho p) e -> p ho e", p=P))
            xTr = xT[:].rearrange("(eo p) c -> p eo c", p=P)
            for eo in range(NE):
                nc.scalar.dma_start(out=x_sb[:, eo, :], in_=xTr[:, eo, :])
            make_identity(nc, ident)
            nc.vector.memset(v_sb, 1.0)   # ones column survives; V copies overwrite the rest
            nc.vector.memset(ones_sb, 1.0)

            # ---- K/V projection: psum = [K.T ; V.T]; eo-outer so x streams ----
            pkv = [ps.tile([P, SW], f32, tag="mm", name=f"pkv{s}") for s in range(NS)]
            for eo in range(NE):
                for s in range(NS):
                    nc.tensor.matmul(
                        pkv[s], wkv_sb[:, eo, :], x_sb[:, eo, s * SW:(s + 1) * SW],
                        start=(eo == 0), stop=(eo == NE - 1))
            nc.vector.memset(kt_e[D:P, :], 0.0)
            nc.vector.memset(kt_o[0:D, :], 0.0)
            for s in range(NS):
                nc.vector.tensor_copy(out=kt_e[0:D, s * SW:(s + 1) * SW], in_=pkv[s][0:D, :])
                nc.vector.tensor_copy(out=vt_sb[D:P, s * SW:(s + 1) * SW], in_=pkv[s][D:P, :])
            # odd-head copy of K.T on the other partition half (zero-padded K=128
            # keeps the score matmuls full-array so the HAM clock stays warm)
            nc.sync.dma_start(out=kt_o[D:P, :], in_=kt_e[0:D, :])
            # transpose V.T -> V natural [s, d] chunks
            for i in range(NQ):
                ptr = psx.tile([P, D], bf16, tag="aux")
                nc.tensor.transpose(ptr, vt_sb[D:P, i * P:(i + 1) * P], ident[D:P, D:P])
                nc.vector.tensor_copy(out=v_sb[:, i, 0:D], in_=ptr)

            def emit_q_strip(m, s):
                """One dense full-array accumulation burst of the Q projection."""
                pq = ps.tile([P, SW], f32, tag="mm", name=f"pq{m}_{s}")
                for eo in range(NE):
                    nc.tensor.matmul(
                        pq, wq_sb[:, eo, m * P:(m + 1) * P],
                        x_sb[:, eo, s * SW:(s + 1) * SW],
                        start=(eo == 0), stop=(eo == NE - 1))
                nc.vector.tensor_copy(out=qt_sb[:, m, s * SW:(s + 1) * SW], in_=pq)

            def emit_wo_group(qcs):
                """Output projection for a few q-chunks, both hd halves."""
                for qc in qcs:
                    o_sb = outs.tile([P, E], f32, tag="o", name=f"o{qc}")
                    for es in range(NS):
                        po = ps.tile([P, SW], f32, tag="mm", name=f"po{qc}_{es}")
                        for m in range(2):
                            nc.tensor.matmul(
                                po, hid_sb[:, m, qc * P:(qc + 1) * P],
                                wo_sb[:, m, es * SW:(es + 1) * SW],
                                start=(m == 0), stop=(m == 1))
                        if es == 3:   # the output tail is DVE-copy-bound; ACT is idle
                            nc.scalar.copy(out=o_sb[:, es * SW:(es + 1) * SW], in_=po)
                        else:
                            nc.vector.tensor_copy(out=o_sb[:, es * SW:(es + 1) * SW], in_=po)
                    nc.sync.dma_start(out=outp[qc * P:(qc + 1) * P, :], in_=o_sb)

            # ---- attention: head pairs, with warm-keeper bursts per strip ----
            rec_tiles = {}
            ht_tiles = {1: htp.tile([D, C], bf16, tag="ht", name="ht1"),
                        3: htp.tile([D, C], bf16, tag="ht", name="ht3")}

            def emit_attention(pair, fillers):
                m = pair
                den = {h: denp.tile([P, C], f32, tag="den", name=f"den{pair}_{h}")
                       for h in (0, 1)}
                for j in range(NS):
                    ph = {h: psh.tile([D + 1, SW], f32, tag="hid", name=f"ph{pair}_{j}_{h}")
                          for h in (0, 1)}
                    for i in range(4 * j + 4):
                        qlo = max(SW * j, P * i)
                        qhi = SW * (j + 1)
                        w = qhi - qlo
                        llo = qlo - SW * j
                        for h in (0, 1):      # zero-padded K=128: full PE rows
                            ktp = kt_e if h == 0 else kt_o
                            psc = ps.tile([P, SW], f32, tag="mm")
                            nc.tensor.matmul(
                                psc[:, :w],
                                ktp[:, i * P:(i + 1) * P],
                                qt_sb[:, m, qlo:qhi],
                                start=True, stop=True)
                            et = expp.tile([P, SW], bf16, tag="exp")
                            nc.scalar.activation(out=et[:, :w], in_=psc[:, :w],
                                                 func=AF.Exp, scale=0.125)
                            if qlo == P * i:   # diagonal block: zero q<s entries
                                nc.vector.tensor_mul(et[:, 0:P], et[:, 0:P], tri_sb)
                            nc.tensor.matmul(
                                ph[h][:, llo:], v_sb[:, i, :], et[:, :w],
                                start=(i == 0), stop=(i == 4 * j + 3),
                                skip_group_check=True)
                    sl = slice(SW * j, SW * (j + 1))
                    for h in (0, 1):
                        # move raw hidden + den out of PSUM quickly
                        if h == 0:
                            nc.vector.tensor_copy(out=hid_sb[0:D, m, sl], in_=ph[h][0:D, :])
                        else:
                            nc.vector.tensor_copy(out=ht_tiles[2 * pair + 1][:, sl],
                                                  in_=ph[h][0:D, :])
                        nc.vector.tensor_copy(out=den[h][D:D + 1, sl], in_=ph[h][D:D + 1, :])
                    if j < len(fillers):
                        fillers[j]()           # dense full-array burst (HAM warm-keeper)
                # 1/den: bounce through DRAM to reshape the 1-lane row into
                # [128, 16] so the DVE reciprocal is free-size-cheap
                for h in (0, 1):
                    hh = 2 * pair + h
                    nc.sync.dma_start(out=scr_den[hh], in_=den[h][D:D + 1, :])
                    dd = d128p.tile([P, NQ], f32, tag="d128", name=f"dd{hh}")
                    nc.sync.dma_start(out=dd, in_=scr_den[hh].rearrange("a (p o) -> (a p) o", p=P))
                    rr = d128p.tile([P, NQ], f32, tag="r128", name=f"rr{hh}")
                    nc.vector.reciprocal(out=rr, in_=dd)
                    nc.sync.dma_start(out=scr_rec[hh].rearrange("a (p o) -> (a p) o", p=P), in_=rr)
                    rec = recp.tile([P, C], bf16, tag="rec", name=f"rec{hh}")
                    rec_tiles[hh] = rec
                    nc.gpsimd.dma_start(out=rec[D:D + 1, :], in_=scr_rec[hh])

            def emit_normalize(hh):
                m, odd = hh // 2, hh % 2
                rec = rec_tiles[hh]
                for s in range(NS):
                    sl = slice(s * SW, (s + 1) * SW)
                    pb = psx.tile([D, SW], f32, tag="aux")
                    nc.tensor.matmul(pb, ones_sb[D:D + 1, :], rec[D:D + 1, sl],
                                     start=True, stop=True)
                    if not odd:
                        nc.vector.tensor_mul(hid_sb[0:D, m, sl], hid_sb[0:D, m, sl], pb)
                    else:
                        ht = ht_tiles[hh]
                        nc.vector.tensor_mul(ht[:, sl], ht[:, sl], pb)
                if odd:
                    nc.sync.dma_start(out=hid_sb[D:P, m, :], in_=ht_tiles[hh])

            for s in range(NS):
                emit_q_strip(0, s)
            # pair 0, with Q m=1 strips as per-strip warm-keepers
            emit_attention(0, [lambda s=s: emit_q_strip(1, s) for s in range(NS)])
            emit_normalize(0)
            emit_normalize(1)
            emit_attention(1, [])
            emit_normalize(2)
            emit_normalize(3)
            emit_wo_group(range(NQ))

    nc.finalize()
    return nc


def make_core_inputs(x, W_q, W_k, W_v, W_o):
    """Host-side shard + pre-transpose + bf16 cast. Returns list of in_maps."""
    x2 = np.ascontiguousarray(x.reshape(C, E).T).astype(BF16)      # [E, C]
    tri_np = np.triu(np.ones((P, P), np.float32)).astype(BF16)     # q>=s valid
    in_maps = []
    for c in range(NCORES):
        qsl = slice(c * HD_C, (c + 1) * HD_C)
        ksl = slice(c * D, (c + 1) * D)
        wq_t = np.ascontiguousarray(W_q[qsl].T).astype(BF16)                    # [E, 256]
        wkv = np.concatenate([W_k[ksl], W_v[ksl]], axis=0)                      # [128, E]
        wkv_t = np.ascontiguousarray(wkv.T).astype(BF16)                        # [E, 128]
        wo_t = np.ascontiguousarray(W_o[:, qsl].T).astype(BF16)                 # [256, E]
        in_maps.append({
            "xT": x2, "wqT": wq_t, "wkvT": wkv_t, "woT": wo_t, "tri": tri_np,
        })
    return in_maps


def kernel(x, W_q, W_k, W_v, W_o):
    global LAST_RESULTS
    from concourse.bass_utils import run_bass_kernel_spmd

    if "nc" not in _CACHE:
        _CACHE["nc"] = build_bass()
    nc = _CACHE["nc"]

    in_maps = make_core_inputs(
        np.asarray(x, np.float32), np.asarray(W_q, np.float32),
        np.asarray(W_k, np.float32), np.asarray(W_v, np.float32),
        np.asarray(W_o, np.float32))

    res = run_bass_kernel_spmd(nc, in_maps, core_ids=list(range(NCORES)), trace=TRACE)
    LAST_RESULTS = res

    out = np.zeros((C, E), np.float32)
    for r in res.results:
        out += r["out_part"]
    return out.reshape(B, C, E)


# revision 23
# speedup vs baseline: 1.0058x; 1.0058x over previous
"""Trainium2 Bass kernel for GQA MultiHeadAttention (nn_MultiHeadAttention_74028056314029).

Reference computation (fp32, single device):
    Q = x @ W_q.T; K = x @ W_k.T; V = x @ W_v.T   (H=32 query heads, KV=8, G=4)
    per query head: softmax(causal(Q_h K_h^T / sqrt(D))) @ V_h
    out = hidden @ W_o.T

Sharding (8 NeuronCores, tensor-parallel over heads):
    core c owns query heads [4c, 4c+4) == KV group c (1 KV head).
    Each core computes a full-shape partial of the output projection;
    the 8 partials are summed on the host - no on-device collective.

Device scheme (bf16 matmuls, fp32 PSUM):
    - host pre-transposes x / weight shards so every DMA is contiguous;
      weights go on the sync HWDGE ring, x chunks on the scalar ring so the
      first projection matmul starts ~4us in
    - Q.T/K.T live [head_dim, seq]; scores come out transposed [s, q]; the
      K.T lhsT is zero-padded to K=128 per head parity ([K.T;0] / [0;K.T]) so
      every score matmul streams the full PE array - half-array K=64 streams
      pin the HAM clock gate at 1.2 GHz
    - exp on ACT with the 1/sqrt(D) scale fused; causal diagonal masked by a
      bf16 upper-tri multiply; attn@V appends a ones column to V so the
      softmax denominator falls out of the same accumulation (row 64)
    - 1/den: the 1-lane den row bounces through DRAM into [128,16] so the DVE
      reciprocal is cheap, then a K=1 ones-matmul broadcasts it across
      partitions; the normalize multiply stays off the PE critical path
    - the Q m=1 projection strips are interleaved into attention pair 0 as
      dense accumulation bursts that keep the HAM clock gate warm
"""

import os
import numpy as np
import ml_dtypes

E, H, KVH, D = 2048, 32, 8, 64
B, C = 1, 2048
G = H // KVH              # 4 query heads per core
NCORES = 8
HD_C = G * D              # 256 query head dims per core
P = 128
NE = E // P               # 16 contraction chunks
NQ = C // P               # 16 sequence chunks
SW = 512                  # strip width (one PSUM bank of fp32)
NS = C // SW              # 4 strips

BF16 = ml_dtypes.bfloat16

_CACHE: dict = {}
LAST_RESULTS = None       # BassKernelResults of the most recent run (for profiling)
TRACE = bool(int(os.environ.get("KERNEL_TRACE", "0")))


def build_bass():
    import concourse.tile as tile
    import concourse.mybir as mybir
    from concourse import bacc
    from concourse.masks import make_identity

    bf16 = mybir.dt.bfloat16
    f32 = mybir.dt.float32
    AF = mybir.ActivationFunctionType

    nc = bacc.Bacc()
    xT = nc.declare_dram_parameter("xT", [E, C], bf16, isOutput=False)
    wqT = nc.declare_dram_parameter("wqT", [E, HD_C], bf16, isOutput=False)
    wkvT = nc.declare_dram_parameter("wkvT", [E, 2 * D], bf16, isOutput=False)
    woT = nc.declare_dram_parameter("woT", [HD_C, E], bf16, isOutput=False)
    tri = nc.declare_dram_parameter("tri", [P, P], bf16, isOutput=False)
    outp = nc.declare_dram_parameter("out_part", [C, E], f32, isOutput=True)
    scr_den = nc.dram_tensor("scr_den", [G, 1, C], f32)
    scr_rec = nc.dram_tensor("scr_rec", [G, 1, C], f32)

    with tile.TileContext(nc) as tc:
        with (
            tc.tile_pool(name="big", bufs=1) as big,
            tc.tile_pool(name="expp", bufs=6) as expp,
            tc.tile_pool(name="denp", bufs=2) as denp,
            tc.tile_pool(name="d128", bufs=2) as d128p,
            tc.tile_pool(name="recp", bufs=3) as recp,
            tc.tile_pool(name="htp", bufs=2) as htp,
            tc.tile_pool(name="outs", bufs=2) as outs,
            tc.tile_pool(name="ps", bufs=4, space="PSUM") as ps,
            tc.tile_pool(name="psx", bufs=2, space="PSUM") as psx,
            tc.tile_pool(name="psh", bufs=2, space="PSUM") as psh,
        ):
            # ---- persistent SBUF tensors ----
            x_sb = big.tile([P, NE, C], bf16)        # x.T: E on partitions
            wq_sb = big.tile([P, NE, HD_C], bf16)
            wkv_sb = big.tile([P, NE, 2 * D], bf16)  # [W_k | W_v] shard, transposed
            wo_sb = big.tile([P, 2, E], bf16)        # W_o shard transposed: hd on partitions
            tri_sb = big.tile([P, P], bf16)          # upper-tri ones (q>=s valid)
            ident = big.tile([P, P], bf16)
            ones_sb = big.tile([P, D], bf16)         # ones row for the K=1 PE broadcast
            kt_e = big.tile([P, C], bf16)            # [K.T ; 0] for even heads
            kt_o = big.tile([P, C], bf16)            # [0 ; K.T] for odd heads
            vt_sb = big.tile([P, C], bf16)           # V.T staged at partitions 64:128
            v_sb = big.tile([P, NQ, D + 1], bf16)    # V natural + ones column
            qt_sb = big.tile([P, 2, C], bf16)        # Q.T: head-dim on partitions
            hid_sb = big.tile([P, 2, C], bf16)       # hidden.T (raw, then normalized)

            # weights first on the sync ring; x chunks stream on the ACT ring
            nc.sync.dma_start(out=wkv_sb, in_=wkvT[:].rearrange("(eo p) m -> p eo m", p=P))
            nc.sync.dma_start(out=wq_sb, in_=wqT[:].rearrange("(eo p) m -> p eo m", p=P))
            nc.sync.dma_start(out=tri_sb, in_=tri[:])
            nc.sync.dma_start(out=wo_sb, in_=woT[:].rearrange("(ho p) e -> p ho e", p=P))
            xTr = xT[:].rearrange("(eo p) c -> p eo c", p=P)
            for eo in range(NE):
                nc.scalar.dma_start(out=x_sb[:, eo, :], in_=xTr[:, eo, :])
            make_identity(nc, ident)
            nc.vector.memset(v_sb, 1.0)   # ones column survives; V copies overwrite the rest
            nc.vector.memset(ones_sb, 1.0)

            # ---- K/V projection: psum = [K.T ; V.T]; eo-outer so x streams ----
            pkv = [ps.tile([P, SW], f32, tag="mm", name=f"pkv{s}") for s in range(NS)]
            for eo in range(NE):
                for s in range(NS):
                    nc.tensor.matmul(
                        pkv[s], wkv_sb[:, eo, :], x_sb[:, eo, s * SW:(s + 1) * SW],
                        start=(eo == 0), stop=(eo == NE - 1))
            nc.vector.memset(kt_e[D:P, :], 0.0)
            nc.vector.memset(kt_o[0:D, :], 0.0)
            for s in range(NS):
                nc.vector.tensor_copy(out=kt_e[0:D, s * SW:(s + 1) * SW], in_=pkv[s][0:D, :])
                nc.vector.tensor_copy(out=vt_sb[D:P, s * SW:(s + 1) * SW], in_=pkv[s][D:P, :])
            # odd-head copy of K.T on the other partition half (zero-padded K=128
            # keeps the score matmuls full-array so the HAM clock stays warm)
            nc.sync.dma_start(out=kt_o[D:P, :], in_=kt_e[0:D, :])
            # transpose V.T -> V natural [s, d] chunks
            for i in range(NQ):
                ptr = psx.tile([P, D], bf16, tag="aux")
                nc.tensor.transpose(ptr, vt_sb[D:P, i * P:(i + 1) * P], ident[D:P, D:P])
                nc.vector.tensor_copy(out=v_sb[:, i, 0:D], in_=ptr)

            def emit_q_strip(m, s):
                """One dense full-array accumulation burst of the Q projection."""
                pq = ps.tile([P, SW], f32, tag="mm", name=f"pq{m}_{s}")
                for eo in range(NE):
                    nc.tensor.matmul(
                        pq, wq_sb[:, eo, m * P:(m + 1) * P],
                        x_sb[:, eo, s * SW:(s + 1) * SW],
                        start=(eo == 0), stop=(eo == NE - 1))
                nc.vector.tensor_copy(out=qt_sb[:, m, s * SW:(s + 1) * SW], in_=pq)

            def emit_wo_group(qcs):
                """Output projection for a few q-chunks, both hd halves."""
                for qc in qcs:
                    o_sb = outs.tile([P, E], f32, tag="o", name=f"o{qc}")
                    for es in range(NS):
                        po = ps.tile([P, SW], f32, tag="mm", name=f"po{qc}_{es}")
                        for m in range(2):
                            nc.tensor.matmul(
                                po, hid_sb[:, m, qc * P:(qc + 1) * P],
                                wo_sb[:, m, es * SW:(es + 1) * SW],
                                start=(m == 0), stop=(m == 1))
                        nc.vector.tensor_copy(out=o_sb[:, es * SW:(es + 1) * SW], in_=po)
                    nc.sync.dma_start(out=outp[qc * P:(qc + 1) * P, :], in_=o_sb)

            # ---- attention: head pairs, with warm-keeper bursts per strip ----
            rec_tiles = {}
            ht_tiles = {1: htp.tile([D, C], bf16, tag="ht", name="ht1"),
                        3: htp.tile([D, C], bf16, tag="ht", name="ht3")}

            def emit_attention(pair, fillers):
                m = pair
                den = {h: denp.tile([P, C], f32, tag="den", name=f"den{pair}_{h}")
                       for h in (0, 1)}
                for j in range(NS):
                    ph = {h: psh.tile([D + 1, SW], f32, tag="hid", name=f"ph{pair}_{j}_{h}")
                          for h in (0, 1)}
                    for i in range(4 * j + 4):
                        qlo = max(SW * j, P * i)
                        qhi = SW * (j + 1)
                        w = qhi - qlo
                        llo = qlo - SW * j
                        ets = {}
                        for h in (0, 1):      # zero-padded K=128: full PE rows
                            ktp = kt_e if h == 0 else kt_o
                            psc = ps.tile([P, SW], f32, tag="mm")
                            nc.tensor.matmul(
                                psc[:, :w],
                                ktp[:, i * P:(i + 1) * P],
                                qt_sb[:, m, qlo:qhi],
                                start=True, stop=True)
                            et = expp.tile([P, SW], bf16, tag="exp")
                            nc.scalar.activation(out=et[:, :w], in_=psc[:, :w],
                                                 func=AF.Exp, scale=0.125)
                            if qlo == P * i:   # diagonal block: zero q<s entries
                                nc.vector.tensor_mul(et[:, 0:P], et[:, 0:P], tri_sb)
                            ets[h] = et
                        for h in (0, 1):      # adjacent attnVs share the V-chunk lhsT
                            nc.tensor.matmul(
                                ph[h][:, llo:], v_sb[:, i, :], ets[h][:, :w],
                                start=(i == 0), stop=(i == 4 * j + 3),
                                skip_group_check=True)
                    sl = slice(SW * j, SW * (j + 1))
                    for h in (0, 1):
                        # move raw hidden + den out of PSUM quickly
                        if h == 0:
                            nc.vector.tensor_copy(out=hid_sb[0:D, m, sl], in_=ph[h][0:D, :])
                        else:
                            nc.vector.tensor_copy(out=ht_tiles[2 * pair + 1][:, sl],
                                                  in_=ph[h][0:D, :])
                        nc.vector.tensor_copy(out=den[h][D:D + 1, sl], in_=ph[h][D:D + 1, :])
                    if j < len(fillers):
                        fillers[j]()           # dense full-array burst (HAM warm-keeper)
                # 1/den: bounce through DRAM to reshape the 1-lane row into
                # [128, 16] so the DVE reciprocal is free-size-cheap
                for h in (0, 1):
                    hh = 2 * pair + h
                    nc.sync.dma_start(out=scr_den[hh], in_=den[h][D:D + 1, :])
                    dd = d128p.tile([P, NQ], f32, tag="d128", name=f"dd{hh}")
                    nc.sync.dma_start(out=dd, in_=scr_den[hh].rearrange("a (p o) -> (a p) o", p=P))
                    rr = d128p.tile([P, NQ], f32, tag="r128", name=f"rr{hh}")
                    nc.vector.reciprocal(out=rr, in_=dd)
                    nc.sync.dma_start(out=scr_rec[hh].rearrange("a (p o) -> (a p) o", p=P), in_=rr)
                    rec = recp.tile([P, C], bf16, tag="rec", name=f"rec{hh}")
                    rec_tiles[hh] = rec
                    nc.gpsimd.dma_start(out=rec[D:D + 1, :], in_=scr_rec[hh])

            def emit_normalize(hh):
                m, odd = hh // 2, hh % 2
                rec = rec_tiles[hh]
                for s in range(NS):
                    sl = slice(s * SW, (s + 1) * SW)
                    pb = psx.tile([D, SW], f32, tag="aux")
                    nc.tensor.matmul(pb, ones_sb[D:D + 1, :], rec[D:D + 1, sl],
                                     start=True, stop=True)
                    if not odd:
                        nc.vector.tensor_mul(hid_sb[0:D, m, sl], hid_sb[0:D, m, sl], pb)
                    else:
                        ht = ht_tiles[hh]
                        nc.vector.tensor_mul(ht[:, sl], ht[:, sl], pb)
                if odd:
                    nc.sync.dma_start(out=hid_sb[D:P, m, :], in_=ht_tiles[hh])

            for s in range(NS):
                emit_q_strip(0, s)
            # pair 0, with Q m=1 strips as per-strip warm-keepers
            emit_attention(0, [lambda s=s: emit_q_strip(1, s) for s in range(NS)])
            emit_normalize(0)
            emit_normalize(1)
            emit_attention(1, [])
            emit_normalize(2)
            emit_normalize(3)
            emit_wo_group(range(NQ))

    nc.finalize()
    return nc


def make_core_inputs(x, W_q, W_k, W_v, W_o):
    """Host-side shard + pre-transpose + bf16 cast. Returns list of in_maps."""
    x2 = np.ascontiguousarray(x.reshape(C, E).T).astype(BF16)      # [E, C]
    tri_np = np.triu(np.ones((P, P), np.float32)).astype(BF16)     # q>=s valid
    in_maps = []
    for c in range(NCORES):
        qsl = slice(c * HD_C, (c + 1) * HD_C)
        ksl = slice(c * D, (c + 1) * D)
        wq_t = np.ascontiguousarray(W_q[qsl].T).astype(BF16)                    # [E, 256]
        wkv = np.concatenate([W_k[ksl], W_v[ksl]], axis=0)                      # [128, E]
        wkv_t = np.ascontiguousarray(wkv.T).astype(BF16)                        # [E, 128]
        wo_t = np.ascontiguousarray(W_o[:, qsl].T).astype(BF16)                 # [256, E]
        in_maps.append({
            "xT": x2, "wqT": wq_t, "wkvT": wkv_t, "woT": wo_t, "tri": tri_np,
        })
    return in_maps


def kernel(x, W_q, W_k, W_v, W_o):
    global LAST_RESULTS
    from concourse.bass_utils import run_bass_kernel_spmd

    if "nc" not in _CACHE:
        _CACHE["nc"] = build_bass()
    nc = _CACHE["nc"]

    in_maps = make_core_inputs(
        np.asarray(x, np.float32), np.asarray(W_q, np.float32),
        np.asarray(W_k, np.float32), np.asarray(W_v, np.float32),
        np.asarray(W_o, np.float32))

    res = run_bass_kernel_spmd(nc, in_maps, core_ids=list(range(NCORES)), trace=TRACE)
    LAST_RESULTS = res

    out = np.zeros((C, E), np.float32)
    for r in res.results:
        out += r["out_part"]
    return out.reshape(B, C, E)
